# revision 18
# baseline (speedup 1.0000x reference)
"""Trainium2 Bass kernel v6 for DirectedGraphConv.

Math (per batch b, node n):
    out = feature + einsum("bni,doi->bno", feature, weights) + bias[graph].sum(axis=2)

Identities:
  * einsum sums over directions d and input dim i:  out_mm = F @ (W0+W1+I)^T
    (+feature folds in as +I, added to the direction-sum on device).
  * bias[graph].sum(axis=2) = Count @ bias.  Counts come from 16 histogram
    matmuls over 16 linearly-independent "plane" functions of the labels:
    11 is_equal indicators (DVE), an all-ones plane, and 4 ReLU ramps
    relu(g - a), a = 11.5..14.5 (ACT engine).  The change of basis back to
    per-label counts is an exact small-integer matrix A folded into the
    matmul selector stationaries (built on-chip, no DMA dependency).

Device does all arithmetic.  Host only reshapes/transposes/casts inputs and
upcasts the bf16 output.

Scheduling: 7 input DMAs across both HWDGE rings (8 sem lanes, no reuse
stalls), warmup matmuls keep the PE clock high until the graph lands, planes
are produced by DVE and ACT in parallel, W-chunk sums interleave into the DVE
stream as chunks land, and only the 4 bias matmuls + copies trail the end.
"""

import numpy as np
import ml_dtypes

BF16 = ml_dtypes.bfloat16

B, N, D = 32, 128, 512
DIR = 2
L = 16
NC = 8
BPC = B // NC  # 4
BN = BPC * N  # 512
P = 128
KC = D // P  # 4
WARMUP_MMS = 42
NEQ = 11  # is_equal planes (labels 0..10)
NRP = 4  # relu ramp planes


def _amatrix():
    # planes: delta_0..10, ones, relu(x-11.5), relu(x-12.5), relu(x-13.5),
    # relu(x-14.5).  counts = A @ plane_sums; A is exactly integral.
    xs = np.arange(L)
    planes = [(xs == l).astype(np.float64) for l in range(NEQ)]
    planes.append(np.ones(L))
    for i in range(NRP):
        planes.append(np.maximum(xs - (NEQ + 0.5 + i), 0.0))
    M = np.stack(planes)  # [16 planes, 16 labels]
    A = np.linalg.inv(M)  # counts = A @ S
    A = np.round(A)
    return A  # A[j, k]: weight of plane k into count row j


_prog_cache: dict = {}


def _build():
    import concourse.bass as bass  # noqa: F401
    import concourse.mybir as mybir
    import concourse.tile as tile
    from concourse import bacc
    from concourse.masks import make_identity

    f32 = mybir.dt.float32
    bf16 = mybir.dt.bfloat16
    u8 = mybir.dt.uint8

    nc = bacc.Bacc("TRN2", target_bir_lowering=False, debug=False, num_devices=NC)

    # Host-prepped layouts (pure relayout/cast of the original inputs):
    #   ft  [128 p, c, bn]    bf16   ft[p,c,b*128+n] = F[b,n,c*128+p]
    #   wt  [128 p, c, d, o]  bf16   wt[p,c,d,o]     = W[d,o,c*128+p]
    #   g   [128 m, bn]       uint8  g[m,b*128+n]    = graph[b,n,m]
    #   cb  [16, 512]         bf16   bias
    ft = nc.dram_tensor("ft", [P, KC, BN], bf16, kind="ExternalInput").ap()
    wt = nc.dram_tensor("wt", [P, KC, DIR, D], bf16, kind="ExternalInput").ap()
    g = nc.dram_tensor("g", [P, BN], u8, kind="ExternalInput").ap()
    cb = nc.dram_tensor("cb", [L, D], bf16, kind="ExternalInput").ap()
    out = nc.dram_tensor("out", [BPC, N, D], bf16, kind="ExternalOutput").ap()

    A = _amatrix()

    with tile.TileContext(nc) as tc:
        with (
            tc.tile_pool(name="work", bufs=1) as wpool,
            tc.tile_pool(name="psum", bufs=1, space="PSUM") as ppool,
        ):
            # ---- DMA triggers first so the rings start moving at t=0 ----
            G_sb = wpool.tile([P, BN], u8)
            nc.scalar.dma_start(out=G_sb, in_=g)
            cb_sb = wpool.tile([L, D], bf16)
            nc.scalar.dma_start(out=cb_sb, in_=cb)
            FT = wpool.tile([P, KC, BN], bf16)
            nc.sync.dma_start(out=FT, in_=ft)
            W_sb = wpool.tile([P, KC, DIR, D], bf16)
            for c in range(KC):
                nc.sync.dma_start(out=W_sb[:, c, :, :], in_=wt[:, c, :, :])

            # ---- on-chip constants (gpsimd, no input deps) ----
            # dummy/ones first: they unblock the PE warmup + ones-matmul;
            # the esel chain can finish later (first count matmul ~12.7us)
            dummy = wpool.tile([P, P], bf16)
            nc.gpsimd.memset(dummy, 1.0)
            ones512 = wpool.tile([P, BN], bf16)
            nc.gpsimd.memset(ones512, 1.0)
            # esel[m, k, j] = A[j, k]: selector stationaries per plane k
            esel = wpool.tile([P, L, L], bf16)
            nc.gpsimd.memset(esel, 0.0)
            esel3 = esel[:, 0:NEQ, :]
            nc.gpsimd.affine_select(
                out=esel3,
                in_=esel3,
                compare_op=mybir.AluOpType.not_equal,
                fill=1.0,
                base=0,
                pattern=[[1, NEQ], [-1, L]],
                channel_multiplier=0,
            )
            # delta planes also contribute -cnt_l to count row 11 (A[11,l]=-1)
            nc.gpsimd.affine_select(
                out=esel3,
                in_=esel3,
                compare_op=mybir.AluOpType.not_equal,
                fill=-1.0,
                base=-NEQ,
                pattern=[[0, NEQ], [1, L]],
                channel_multiplier=0,
            )
            for k in range(NEQ, L):
                for j in range(NEQ, L):
                    v = float(A[j, k])
                    if v != 0.0:
                        nc.gpsimd.memset(esel[:, k, j : j + 1], v)
            ib_sb = wpool.tile([P, P], bf16)
            make_identity(nc, ib_sb)
            rpb = wpool.tile([P, NRP], f32)
            for i in range(NRP):
                nc.gpsimd.memset(rpb[:, i : i + 1], -(NEQ + 0.5 + i))

            # ACT table preload (self-copy, no deps)
            act_warm = wpool.tile([P, 2], f32)
            nc.scalar.copy(out=act_warm[:, 0:1], in_=act_warm[:, 1:2])

            # ---- HAM warmup ----
            psum_warm = ppool.tile([P, P], f32, tag="warm", bufs=1)
            for _ in range(WARMUP_MMS):
                nc.tensor.matmul(
                    out=psum_warm, lhsT=dummy, rhs=dummy, start=True, stop=True
                )

            # ---- DVE stream: W chunk 0 sum, graph cast, EQ planes with the
            # remaining chunk sums interleaved as their DMAs land ----
            Wsum = wpool.tile([P, KC, D], bf16)

            def _wsum(c):
                nc.vector.tensor_tensor(
                    out=Wsum[:, c, :],
                    in0=W_sb[:, c, 0, :],
                    in1=W_sb[:, c, 1, :],
                    op=mybir.AluOpType.add,
                )
                sl = slice(c * P, (c + 1) * P)
                nc.vector.tensor_tensor(
                    out=Wsum[:, c, sl],
                    in0=Wsum[:, c, sl],
                    in1=ib_sb,
                    op=mybir.AluOpType.add,
                )

            _wsum(0)
            G_bf = wpool.tile([P, BN], bf16)
            nc.vector.tensor_copy(out=G_bf, in_=G_sb)
            EQ = wpool.tile([P, NEQ, BN], bf16)
            wsum_after = {2: 1, 5: 2, 7: 3}
            for l in range(NEQ):
                nc.vector.tensor_scalar(
                    out=EQ[:, l, :],
                    in0=G_bf,
                    scalar1=float(l),
                    scalar2=None,
                    op0=mybir.AluOpType.is_equal,
                )
                if l in wsum_after:
                    _wsum(wsum_after[l])

            # ---- ACT stream: relu ramp planes ----
            RP = wpool.tile([P, NRP, BN], bf16)
            for i in range(NRP):
                nc.scalar.activation(
                    out=RP[:, i, :],
                    in_=G_bf,
                    func=mybir.ActivationFunctionType.Relu,
                    bias=rpb[:, i : i + 1],
                    scale=1.0,
                )

            # ---- histogram matmuls + main matmuls, interleaved on the PE ----
            psum_cnt = ppool.tile([L, BN], f32, tag="cnt", bufs=1)
            psum_outs = [
                ppool.tile([P, D], f32, tag=f"out{b}", bufs=1, name=f"psum_out{b}")
                for b in range(BPC)
            ]

            def cnt_mm(slot, rhs, start=False, stop=False):
                nc.tensor.matmul(
                    out=psum_cnt,
                    lhsT=esel[:, slot, :],
                    rhs=rhs,
                    start=start,
                    stop=stop,
                )

            def mains(c, start=False):
                for b in range(BPC):
                    nc.tensor.matmul(
                        out=psum_outs[b],
                        lhsT=FT[:, c, b * P : (b + 1) * P],
                        rhs=Wsum[:, c, :],
                        start=start,
                        stop=False,
                    )

            cnt_mm(NEQ, ones512, start=True)  # ones plane, slot 11
            # filler warmups: keep the PE busy while the graph-derived
            # planes are still in flight (harmless if planes arrive early)
            for _ in range(12):
                nc.tensor.matmul(
                    out=psum_warm, lhsT=dummy, rhs=dummy, start=True, stop=True
                )
            for l in range(3):
                cnt_mm(l, EQ[:, l, :])
            mains(0, start=True)
            for l in range(3, 5):
                cnt_mm(l, EQ[:, l, :])
            for i in range(2):
                cnt_mm(L - NRP + i, RP[:, i, :])
            mains(1)
            for l in range(5, 8):
                cnt_mm(l, EQ[:, l, :])
            for i in range(2, 4):
                cnt_mm(L - NRP + i, RP[:, i, :])
            for l in range(8, NEQ):
                cnt_mm(l, EQ[:, l, :], stop=(l == NEQ - 1))
            cntT = wpool.tile([L, BN], bf16)
            nc.scalar.copy(out=cntT, in_=psum_cnt)
            mains(2)

            mains(3)

            # bias matmuls close each output bank; copies + 2 output DMAs
            out_sb = wpool.tile([P, BPC, D], bf16)
            h = D // 2
            for b in range(BPC):
                nc.tensor.matmul(
                    out=psum_outs[b],
                    lhsT=cntT[:, b * P : (b + 1) * P],
                    rhs=cb_sb,
                    start=False,
                    stop=True,
                )
                nc.vector.tensor_copy(out=out_sb[:, b, 0:h], in_=psum_outs[b][:, 0:h])
                nc.scalar.copy(out=out_sb[:, b, h:D], in_=psum_outs[b][:, h:D])
                # both output DMAs on the sync ring: it is idle after the
                # input streams, while the scalar queue is busy with copies
                if b == 1:
                    nc.sync.dma_start(
                        out=out[0:2].rearrange("b n d -> n b d"),
                        in_=out_sb[:, 0:2, :],
                    )
                elif b == 3:
                    nc.sync.dma_start(
                        out=out[2:4].rearrange("b n d -> n b d"),
                        in_=out_sb[:, 2:4, :],
                    )

    nc.compile()
    return nc


def _get_prog():
    if "v6" not in _prog_cache:
        _prog_cache["v6"] = _build()
    return _prog_cache["v6"]


def _shard_inputs(feature, graph, weights, bias):
    f = np.asarray(feature, dtype=np.float32)
    g8 = np.asarray(graph).astype(np.uint8)
    w = np.asarray(weights, dtype=np.float32)
    b = np.asarray(bias, dtype=np.float32).astype(BF16)
    # wt[p, c, d, o] = w[d, o, c*128+p]   (replicated)
    wt = np.ascontiguousarray(
        w.transpose(2, 0, 1).reshape(KC, P, DIR, D).transpose(1, 0, 2, 3)
    ).astype(BF16)
    in_maps = []
    for core in range(NC):
        sl = slice(core * BPC, (core + 1) * BPC)
        fc = f[sl]  # [BPC, N, D]
        ftc = np.ascontiguousarray(
            fc.transpose(2, 0, 1).reshape(KC, P, BN).transpose(1, 0, 2)
        ).astype(BF16)  # [p, c, bn]
        gc = np.ascontiguousarray(
            g8[sl].transpose(2, 0, 1).reshape(P, BN)
        )  # [m, b*128+n]
        in_maps.append({"ft": ftc, "wt": wt, "g": gc, "cb": b})
    return in_maps


def _run(feature, graph, weights, bias, trace=False):
    from concourse.bass_utils import run_bass_kernel_spmd

    in_maps = _shard_inputs(feature, graph, weights, bias)
    nc = _get_prog()
    res = run_bass_kernel_spmd(nc, in_maps, core_ids=list(range(NC)), trace=trace)
    out = np.concatenate(
        [r["out"].astype(np.float32) for r in res.results], axis=0
    )
    return out, res


def kernel(feature, graph, weights, bias):
    out, _ = _run(feature, graph, weights, bias, trace=False)
    return out
